# revision 3
# baseline (speedup 1.0000x reference)
"""Single-head attention (B=4, S=2048, D=E=1024) on 8 trn2 NeuronCores.

Sharding: data-parallel over (batch, q-half) -> 8 shards. Each core: 1024-row
q shard + full 2048 keys of its batch (K/V projections recomputed on both
cores of a batch pair; zero collectives).

Layout: all matmul operands bf16 (host-cast) -> halves DMA traffic and
lets kp/vp/qp/ow live in SBUF simultaneously (no re-streaming, no DRAM
bounce).  Weight tiles are per-t (fine-grained deps) in two alternating
pools (wq->wv, wk->ow) so the next phase's weights prefetch during the
previous phase without SBUF aliasing stalls.  Output-tile-outer loops with
rotating PSUM banks keep drain engines (ACT/DVE) pipelined behind the PE;
softmax sums accumulate on DVE (one ones-matmul per q-block); ctx
accumulation is split into half-key groups so it never waits for the last
exp.  Warmup matmuls keep the PE p-state ramp off the critical path.

Per-core math (all "T" tensors token-transposed on host):
  qp^T [E,q]  = (lhsT=wq[D,E], rhs=qT[D,q]) * (1/sqrt E) + bq/sqrt(E)
  kp^T [E,k]  = (lhsT=wk, rhs=kT) + bk
  vp   [k,E]  = (lhsT=vT[D,k], rhs=wv[D,E]) + bv
  lgT  [k,q]  = (lhsT=kp slice, rhs=qp slice)        (scale folded into qp)
  expT [k,q]  = Exp(lgT + mask*NEG)                  (ACT, per-partition bias)
  s    [.,q]  = ones-matmul over DVE-accumulated exp sums
  ctx^T[E,q]  = (lhsT=vp slice, rhs=expT) * recip(s)
  out  [q,D]  = (lhsT=ctx^T slice, rhs=ow[E,D]) + ob
"""

import os
import numpy as np

P = 128
NEG = -1.0e9
N_WARM = 20


def build_nc(D=1024, E=1024, SK=2048, QSH=1024, QB=512):
    import concourse.bass as bass
    import concourse.mybir as mybir
    import concourse.tile as tile
    from concourse import bacc

    f32 = mybir.dt.float32
    f32r = mybir.dt.float32r
    bf16 = mybir.dt.bfloat16
    AF = mybir.ActivationFunctionType

    DT = D // P           # contraction tiles over model dim (8)
    ET = E // P           # enc tiles (8)
    KT = SK // P          # key tiles (16)
    NQB = QSH // QB       # q blocks (2)
    NKB = SK // QB        # key blocks of 512 (4)
    MQ = QB // P          # q subtiles per block (4)
    ND = D // QB          # out col blocks (2)
    HKT = KT // 2         # key tiles per half (8)
    ISCALE = 1.0 / float(np.sqrt(E))

    nc = bacc.Bacc(trn_type="TRN2")

    # ---- I/O ----
    qT = nc.dram_tensor("qT", [D, QSH], bf16, kind="ExternalInput")[:, :]
    kT = nc.dram_tensor("kT", [D, SK], bf16, kind="ExternalInput")[:, :]
    vT = nc.dram_tensor("vT", [D, SK], bf16, kind="ExternalInput")[:, :]
    wq = nc.dram_tensor("wq", [D, E], bf16, kind="ExternalInput")[:, :]
    wk = nc.dram_tensor("wk", [D, E], bf16, kind="ExternalInput")[:, :]
    wv = nc.dram_tensor("wv", [D, E], bf16, kind="ExternalInput")[:, :]
    ow = nc.dram_tensor("ow", [E, D], bf16, kind="ExternalInput")[:, :]
    mask_cols = nc.dram_tensor("mask_cols", [P, KT], f32, kind="ExternalInput")[:, :]
    ones_d = nc.dram_tensor("ones_d", [P, P], f32r, kind="ExternalInput")[:, :]
    bq_col = nc.dram_tensor("bq_col", [P, ET], f32, kind="ExternalInput")[:, :]
    bk_col = nc.dram_tensor("bk_col", [P, ET], f32, kind="ExternalInput")[:, :]
    bv_bc = nc.dram_tensor("bv_bc", [P, E], f32, kind="ExternalInput")[:, :]
    ob_bc = nc.dram_tensor("ob_bc", [P, D], f32, kind="ExternalInput")[:, :]
    out = nc.dram_tensor("out", [QSH, D], f32, kind="ExternalOutput")[:, :]

    qT_r = qT.rearrange("(t p) n -> p t n", p=P)   # [128, DT, QSH]
    kT_r = kT.rearrange("(t p) n -> p t n", p=P)
    vT_r = vT.rearrange("(t p) n -> p t n", p=P)
    wq_r = wq.rearrange("(t p) n -> p t n", p=P)   # [128, DT, E]
    wk_r = wk.rearrange("(t p) n -> p t n", p=P)
    wv_r = wv.rearrange("(t p) n -> p t n", p=P)
    ow_r = ow.rearrange("(t p) n -> p t n", p=P)   # [128, ET, D]

    def mm(ps, lhsT, rhs, start, stop):
        nc.tensor.matmul(ps, lhsT, rhs, start=start, stop=stop)

    with tile.TileContext(nc) as tc:
        # ---- persistent pools (entered in reverse order of death; LIFO) ----
        smalls_cm = tc.tile_pool(name="smalls", bufs=1)
        smalls = smalls_cm.__enter__()
        osb_cm = tc.tile_pool(name="osb", bufs=3)
        osb = osb_cm.__enter__()
        ctxp_cm = tc.tile_pool(name="ctxp", bufs=2)
        ctxp = ctxp_cm.__enter__()
        expp_cm = tc.tile_pool(name="expp", bufs=1)
        expp = expp_cm.__enter__()
        vpp_cm = tc.tile_pool(name="vpp", bufs=1)
        vpp = vpp_cm.__enter__()
        kpp_cm = tc.tile_pool(name="kpp", bufs=1)
        kpp = kpp_cm.__enter__()
        qpp_cm = tc.tile_pool(name="qpp", bufs=1)
        qpp = qpp_cm.__enter__()
        # alternating weight pools: wA holds wq then wv, wB holds wk then ow.
        wA_cm = tc.tile_pool(name="wA", bufs=1)
        wA = wA_cm.__enter__()
        wB_cm = tc.tile_pool(name="wB", bufs=1)
        wB = wB_cm.__enter__()
        # stream pools: S_A holds qT blocks then vT blocks, S_B holds kT.
        sA_cm = tc.tile_pool(name="sA", bufs=4)
        sA = sA_cm.__enter__()
        sB_cm = tc.tile_pool(name="sB", bufs=4)
        sB = sB_cm.__enter__()

        # one global PSUM pool: 8 bank tags, allocation order chosen so each
        # phase's first bank aliases the earliest-released predecessor.
        gps_cm = tc.tile_pool(name="gps", bufs=1, space="PSUM")
        gps = gps_cm.__enter__()

        # smalls on gpsimd SWDGE: keeps the HWDGE free for the wq stream.
        # Issued after the first qT blocks (Pool program order).
        qp = qpp.tile([P, ET, QSH], bf16, name="qp")
        kp = kpp.tile([P, ET, SK], bf16, name="kp")
        vp = vpp.tile([P, KT, E], bf16, name="vp")

        # wq per-t tiles (fine-grained: QP matmuls start on first arrival);
        # t0 lands as two separate half-tiles so the very first matmuls
        # only wait for a 128KB transfer
        wq_ts = []
        qt01 = sA.tile([P, 4, QB], bf16, tag="s4", name="qt01")
        wq0a = wA.tile([P, E // 2], bf16, tag="w0a", name="wq0a")
        nc.sync.dma_start(wq0a[:], wq_r[:, 0, :E // 2])
        wq0b = wA.tile([P, E // 2], bf16, tag="w0b", name="wq0b")
        nc.sync.dma_start(wq0b[:], wq_r[:, 0, E // 2:])
        wq_ts.append((wq0a, wq0b))
        for t in range(1, DT):
            w = wA.tile([P, E], bf16, tag=f"w{t}", name=f"wq{t}")
            nc.sync.dma_start(w[:], wq_r[:, t, :])
            wq_ts.append(w)
            if t == 3:
                nc.sync.dma_start(qt01[:], qT_r[:, 4:8, 0:QB])
        wk_ts = []
        for t in range(DT):
            w = wB.tile([P, E], bf16, tag=f"w{t}", name=f"wk{t}")
            nc.sync.dma_start(w[:], wk_r[:, t, :])
            wk_ts.append(w)
        # qh0 tb0 as per-t tiles (earliest possible first matmul), rest blocks
        qt0_ts = []
        for ti in range(4):
            qt = sA.tile([P, QB], bf16, tag="s", name=f"qt00_{ti}")
            nc.gpsimd.dma_start(qt[:], qT_r[:, ti, 0:QB])
            qt0_ts.append(qt)
        bq_t = smalls.tile([P, ET], f32, name="bqc")
        nc.gpsimd.dma_start(bq_t[:], bq_col)
        qt1_bs = []
        for tb in range(2):
            qb_t = sA.tile([P, 4, QB], bf16, tag="s4", name=f"qt1_{tb}")
            nc.gpsimd.dma_start(qb_t[:], qT_r[:, 4 * tb:4 * tb + 4, QB:2 * QB])
            qt1_bs.append(qb_t)
        ones_t = smalls.tile([P, P], f32r, name="ones")
        nc.gpsimd.dma_start(ones_t[:], ones_d)
        mask_t = smalls.tile([P, KT], f32, name="maskc")
        nc.gpsimd.dma_start(mask_t[:], mask_cols)
        bk_t = smalls.tile([P, ET], f32, name="bkc")
        nc.gpsimd.dma_start(bk_t[:], bk_col)
        bv_t = smalls.tile([P, E], f32, name="bvc")
        nc.gpsimd.dma_start(bv_t[:], bv_bc)
        ob_t = smalls.tile([P, D], f32, name="obc")
        nc.gpsimd.dma_start(ob_t[:], ob_bc)
        recip_ts = [smalls.tile([P, QB], f32, name=f"recip{i}") for i in range(NQB)]
        sacc_ts = [smalls.tile([P, QB], f32r, name=f"sacc{i}") for i in range(NQB)]

        # ---- warmup: keep the PE p-state ramp off the critical path ----
        with tc.tile_pool(name="warm", bufs=1) as wrm:
            dummy = wrm.tile([P, P], bf16, name="dummy")
            nc.vector.memset(dummy[:], 0.0)
            wps = gps.tile([P, P], f32, tag="b7", name="wps")
            for _ in range(N_WARM):
                mm(wps[:], dummy[:], dummy[:], True, True)

        # ---- QP: qp^T [E, QSH]; loop orders matched to DMA arrival ----
        # qh0 t0-3: t-outer (each arriving wq tile unlocks 8 matmuls);
        # qh0 t4-6: t-outer; t7: m-outer with stops so drains pipeline.
        def wq_sl(t, m):
            if t == 0:
                half = wq_ts[0][0] if m < 4 else wq_ts[0][1]
                return half[:, (m % 4) * P:(m % 4 + 1) * P]
            return wq_ts[t][:, m * P:(m + 1) * P]

        pss0 = [gps.tile([P, QB], f32, tag=f"b{m}", name=f"qpps0_{m}")
                for m in range(ET)]
        for ti in range(4):
            for m in range(ET):
                mm(pss0[m][:], wq_sl(ti, m), qt0_ts[ti][:], ti == 0, False)
        for ti in range(3):
            t = 4 + ti
            for m in range(ET):
                mm(pss0[m][:], wq_sl(t, m), qt01[:, ti, :], False, False)
        for m in range(ET):
            mm(pss0[m][:], wq_sl(7, m), qt01[:, 3, :], False, True)
            nc.scalar.activation(qp[:, m, 0:QB], pss0[m][:], AF.Identity,
                                 bias=bq_t[:, m:m + 1], scale=ISCALE)
        # qh1: m-outer chains (all data resident), banks reused as drained
        for m in range(ET):
            ps = gps.tile([P, QB], f32, tag=f"b{m}", name=f"qpps1_{m}")
            for t in range(DT):
                mm(ps[:], wq_sl(t, m),
                   qt1_bs[t // 4][:, t % 4, :], t == 0, t == DT - 1)
            nc.scalar.activation(qp[:, m, QB:2 * QB], ps[:], AF.Identity,
                                 bias=bq_t[:, m:m + 1], scale=ISCALE)

        # ---- KP: kp^T [E, SK] (kT streamed per key block; m-outer chains) ----
        kcnt = 0
        for nb in range(NKB):
            kt_bs = []
            for tb in range(2):
                kt_b = sB.tile([P, 4, QB], bf16, tag="s", name=f"kt{nb}_{tb}")
                nc.gpsimd.dma_start(
                    kt_b[:], kT_r[:, 4 * tb:4 * tb + 4, nb * QB:(nb + 1) * QB])
                kt_bs.append(kt_b)
            for m in range(ET):
                ps = gps.tile([P, QB], f32, tag=f"b{kcnt % 3}",
                              name=f"kpps{nb}_{m}")
                kcnt += 1
                for t in range(DT):
                    mm(ps[:], wk_ts[t][:, m * P:(m + 1) * P],
                       kt_bs[t // 4][:, t % 4, :], t == 0, t == DT - 1)
                nc.scalar.activation(kp[:, m, nb * QB:(nb + 1) * QB],
                                     ps[:], AF.Identity,
                                     bias=bk_t[:, m:m + 1])

        # wv reuses wA slots (freed as QP's last reads complete); prefetches
        # during KP.  ow reuses wB slots; prefetches during VP.
        wv0a = wA.tile([P, E // 2], bf16, tag="w0a", name="wv0a")
        nc.sync.dma_start(wv0a[:], wv_r[:, 0, :E // 2])
        wv0b = wA.tile([P, E // 2], bf16, tag="w0b", name="wv0b")
        nc.sync.dma_start(wv0b[:], wv_r[:, 0, E // 2:])
        wv_ts = [(wv0a, wv0b)]
        for t in range(1, DT):
            w = wA.tile([P, E], bf16, tag=f"w{t}", name=f"wv{t}")
            nc.sync.dma_start(w[:], wv_r[:, t, :])
            wv_ts.append(w)

        def wv_sl(t, h):
            if t == 0:
                return wv_ts[0][h][:]
            return wv_ts[t][:, h * QB:(h + 1) * QB]

        # ---- VP: vp [SK, E] (vT streamed per key block; tile-outer chains) ----
        vcnt = 0
        for nb in range(NKB):
            vt_bs = []
            for tb in range(2):
                vt_b = sA.tile([P, 4, QB], bf16, tag="s4", name=f"vt{nb}_{tb}")
                nc.sync.dma_start(
                    vt_b[:], vT_r[:, 4 * tb:4 * tb + 4, nb * QB:(nb + 1) * QB])
                vt_bs.append(vt_b)
            for kbl in range(MQ):
                kb = nb * MQ + kbl
                for h in range(2):
                    ps = gps.tile([P, QB], f32, tag=f"b{3 + vcnt % 3}",
                                  name=f"vpps{kb}_{h}")
                    vcnt += 1
                    for t in range(DT):
                        mm(ps[:], vt_bs[t // 4][:, t % 4, kbl * P:(kbl + 1) * P],
                           wv_sl(t, h), t == 0, t == DT - 1)
                    nc.vector.tensor_add(vp[:, kb, h * QB:(h + 1) * QB],
                                         ps[:], bv_t[:, h * QB:(h + 1) * QB])

        ow_ts = []
        for e in range(ET):
            w = wB.tile([P, D], bf16, tag=f"w{e}", name=f"ow{e}")
            nc.sync.dma_start(w[:], ow_r[:, e, :])
            ow_ts.append(w)

        # ---- per q-block: logits/exp -> ctx -> out ----
        for qb in range(NQB):
            q0 = qb * QB
            expa = expp.tile([P, HKT, QB], bf16, tag="expa", name=f"expa{qb}")
            expb = expp.tile([P, HKT, QB], bf16, tag="expb", name=f"expb{qb}")
            exp_half = [expa, expb]
            # ctx in two halves so the out phase never waits for the last
            # normalize (whole-tile dependency granularity)
            ctxa = ctxp.tile([P, 4, QB], bf16, tag="ctxa", name=f"ctxa{qb}")
            ctxb = ctxp.tile([P, 4, QB], bf16, tag="ctxb", name=f"ctxb{qb}")
            sacc = sacc_ts[qb]

            # logits + exp; DVE accumulates softmax sums
            for kb in range(KT):
                ps = gps.tile([P, QB], f32, tag=f"b{kb % 3}", name=f"lg{qb}_{kb}")
                for e in range(ET):
                    mm(ps[:], kp[:, e, kb * P:(kb + 1) * P],
                       qp[:, e, q0:q0 + QB], e == 0, e == ET - 1)
                ex = exp_half[kb // HKT]
                nc.scalar.activation(ex[:, kb % HKT, :], ps[:], AF.Exp,
                                     bias=mask_t[:, kb:kb + 1])
                if kb == 0:
                    nc.vector.tensor_copy(sacc[:], ex[:, 0, :])
                else:
                    nc.vector.tensor_add(sacc[:], sacc[:],
                                         ex[:, kb % HKT, :])

            # ctx accumulation in half-key groups: the first group only
            # needs the first 8 exp tiles, so the PE never waits for the
            # last exp.  e-groups of 4 keep PSUM use at 4 banks (b3-b6).
            cps = {}
            for eg in range(2):
                es = range(4 * eg, 4 * eg + 4)
                for half in range(2):
                    ex = exp_half[half]
                    for e in es:
                        if half == 0:
                            cps[e] = gps.tile([P, QB], f32, tag=f"b{3 + e % 4}",
                                              name=f"c{qb}_{e}")
                        for ki in range(HKT):
                            kb = half * HKT + ki
                            mm(cps[e][:], vp[:, kb, e * P:(e + 1) * P],
                               ex[:, ki, :], kb == 0, kb == KT - 1)
                    if eg == 0 and half == 0:
                        # softmax denominator: ones-matmul + reciprocal
                        s_ps = gps.tile([P, QB], f32, tag="b7",
                                        name=f"sps{qb}")
                        mm(s_ps[:], ones_t[:], sacc[:], True, True)
                        nc.vector.reciprocal(recip_ts[qb][:], s_ps[:])
                    if half == 1:
                        for e in es:
                            ctx_half = ctxa if e < 4 else ctxb
                            nc.vector.tensor_mul(ctx_half[:, e % 4, :],
                                                 cps[e][:], recip_ts[qb][:])

            # out block: out[q0:q0+QB, :] = ctx^T.T @ ow + ob
            last_qb = qb == NQB - 1
            ocnt = 0
            for mq in range(MQ):
                for nd in range(ND):
                    rows = slice(q0 + mq * P, q0 + (mq + 1) * P)
                    last_tile = last_qb and mq == MQ - 1 and nd == ND - 1
                    if last_tile:
                        # final tile as two N=256 chains: the first half's
                        # store overlaps the second half's matmuls, so the
                        # kernel tail is one short store chain
                        for sub in range(2):
                            c0 = nd * QB + sub * (QB // 2)
                            ps = gps.tile([P, QB // 2], f32,
                                          tag=f"b{ocnt % 3}",
                                          name=f"o{qb}_{mq}_{nd}_{sub}")
                            ocnt += 1
                            for e in range(ET):
                                ctx_half = ctxa if e < 4 else ctxb
                                mm(ps[:],
                                   ctx_half[:, e % 4, mq * P:(mq + 1) * P],
                                   ow_ts[e][:, c0:c0 + QB // 2],
                                   e == 0, e == ET - 1)
                            ot = osb.tile([P, QB // 2], f32, tag="oth",
                                          name=f"ot{qb}_{mq}_{nd}_{sub}")
                            nc.vector.tensor_add(ot[:], ps[:],
                                                 ob_t[:, c0:c0 + QB // 2])
                            nc.sync.dma_start(out[rows, c0:c0 + QB // 2],
                                              ot[:])
                        continue
                    ps = gps.tile([P, QB], f32, tag=f"b{ocnt % 3}",
                                  name=f"o{qb}_{mq}_{nd}")
                    ocnt += 1
                    for e in range(ET):
                        ctx_half = ctxa if e < 4 else ctxb
                        mm(ps[:], ctx_half[:, e % 4, mq * P:(mq + 1) * P],
                           ow_ts[e][:, nd * QB:(nd + 1) * QB],
                           e == 0, e == ET - 1)
                    ot = osb.tile([P, QB], f32, tag="ot",
                                  name=f"ot{qb}_{mq}_{nd}")
                    nc.vector.tensor_add(ot[:], ps[:],
                                         ob_t[:, nd * QB:(nd + 1) * QB])
                    eng = (nc.sync if last_qb and mq == MQ - 1 else nc.gpsimd)
                    eng.dma_start(out[rows, nd * QB:(nd + 1) * QB], ot[:])

        gps_cm.__exit__(None, None, None)
        sB_cm.__exit__(None, None, None)
        sA_cm.__exit__(None, None, None)
        wB_cm.__exit__(None, None, None)
        wA_cm.__exit__(None, None, None)
        qpp_cm.__exit__(None, None, None)
        kpp_cm.__exit__(None, None, None)
        vpp_cm.__exit__(None, None, None)
        expp_cm.__exit__(None, None, None)
        ctxp_cm.__exit__(None, None, None)
        osb_cm.__exit__(None, None, None)
        smalls_cm.__exit__(None, None, None)

    nc.compile()
    return nc


def make_in_maps(v, k, q, mask, wq_w, wq_b, wk_w, wk_b, wv_w, wv_b, out_w, out_b,
                 n_cores=8, D=1024, E=1024, SK=2048, QSH=1024):
    """Host-side shard + layout prep (pure data movement + dtype cast)."""
    import ml_dtypes
    bf = ml_dtypes.bfloat16
    ET = E // P
    KT = SK // P
    f = np.float32
    iscale = f(1.0 / np.sqrt(E))
    wq_bf = np.ascontiguousarray(np.asarray(wq_w, f).astype(bf))
    wk_bf = np.ascontiguousarray(np.asarray(wk_w, f).astype(bf))
    wv_bf = np.ascontiguousarray(np.asarray(wv_w, f).astype(bf))
    ow_bf = np.ascontiguousarray(np.asarray(out_w, f).astype(bf))
    bq_col = np.ascontiguousarray((np.asarray(wq_b, f) * iscale).reshape(ET, P).T)
    bk_col = np.ascontiguousarray(np.asarray(wk_b, f).reshape(ET, P).T)
    bv_bc = np.ascontiguousarray(np.broadcast_to(np.asarray(wv_b, f), (P, E)))
    ob_bc = np.ascontiguousarray(
        np.broadcast_to(np.asarray(out_b, f), (P, len(out_b))))
    ones_arr = np.ones((P, P), f)
    in_maps = []
    for c in range(n_cores):
        b, h = divmod(c, 2)
        qTc = np.ascontiguousarray(
            np.asarray(q[b, h * QSH:(h + 1) * QSH, :], f).T.astype(bf))
        kTc = np.ascontiguousarray(np.asarray(k[b], f).T.astype(bf))
        vTc = np.ascontiguousarray(np.asarray(v[b], f).T.astype(bf))
        mc = np.ascontiguousarray(
            (np.asarray(mask[b, 0], f) * np.float32(NEG)).reshape(KT, P).T)
        in_maps.append(dict(qT=qTc, kT=kTc, vT=vTc, mask_cols=mc,
                            ones_d=ones_arr,
                            wq=wq_bf, wk=wk_bf, wv=wv_bf, ow=ow_bf,
                            bq_col=bq_col, bk_col=bk_col,
                            bv_bc=bv_bc, ob_bc=ob_bc))
    return in_maps


_NC_CACHE = {}


def kernel(v, k, q, mask, wq_w, wq_b, wk_w, wk_b, wv_w, wv_b, out_w, out_b):
    from concourse.bass_utils import run_bass_kernel_spmd

    B, S, D = 4, 2048, 1024
    E, QSH = 1024, 1024
    if "nc" not in _NC_CACHE:
        _NC_CACHE["nc"] = build_nc(D=D, E=E, SK=S, QSH=QSH, QB=512)
    nc = _NC_CACHE["nc"]

    in_maps = make_in_maps(v, k, q, mask, wq_w, wq_b, wk_w, wk_b, wv_w, wv_b,
                           out_w, out_b, n_cores=8, D=D, E=E, SK=S, QSH=QSH)
    trace = bool(int(os.environ.get("BASS_KERNEL_TRACE", "0")))
    res = run_bass_kernel_spmd(nc, in_maps, core_ids=list(range(8)), trace=trace)
    if trace:
        print(f"HW exec time: {res.exec_time_ns} ns")
        _NC_CACHE["last_exec_time_ns"] = res.exec_time_ns
        _NC_CACHE["last_trace"] = res.instructions_and_trace

    outp = np.empty((B, S, D), np.float32)
    for c in range(8):
        b, h = divmod(c, 2)
        outp[b, h * QSH:(h + 1) * QSH, :] = res.results[c]["out"]
    return outp


# revision 5
# speedup vs baseline: 1.0010x; 1.0010x over previous
"""Single-head attention (B=4, S=2048, D=E=1024) on 8 trn2 NeuronCores.

Sharding: data-parallel over (batch, q-half) -> 8 shards. Each core: 1024-row
q shard + full 2048 keys of its batch (K/V projections recomputed on both
cores of a batch pair; zero collectives).

Layout: all matmul operands fp16 (host-cast) -> halves DMA traffic and
lets kp/vp/qp/ow live in SBUF simultaneously (no re-streaming, no DRAM
bounce).  Weight tiles are per-t (fine-grained deps) in two alternating
pools (wq->wv, wk->ow) so the next phase's weights prefetch during the
previous phase without SBUF aliasing stalls.  Output-tile-outer loops with
rotating PSUM banks keep drain engines (ACT/DVE) pipelined behind the PE;
softmax sums accumulate on DVE (one ones-matmul per q-block); ctx
accumulation is split into half-key groups so it never waits for the last
exp.  Warmup matmuls keep the PE p-state ramp off the critical path.

Per-core math (all "T" tensors token-transposed on host):
  qp^T [E,q]  = (lhsT=wq[D,E], rhs=qT[D,q]) * (1/sqrt E) + bq/sqrt(E)
  kp^T [E,k]  = (lhsT=wk, rhs=kT) + bk
  vp   [k,E]  = (lhsT=vT[D,k], rhs=wv[D,E]) + bv
  lgT  [k,q]  = (lhsT=kp slice, rhs=qp slice)        (scale folded into qp)
  expT [k,q]  = Exp(lgT + mask*NEG)                  (ACT, per-partition bias)
  s    [.,q]  = ones-matmul over DVE-accumulated exp sums
  ctx^T[E,q]  = (lhsT=vp slice, rhs=expT) * recip(s)
  out  [q,D]  = (lhsT=ctx^T slice, rhs=ow[E,D]) + ob
"""

import os
import numpy as np

P = 128
NEG = -1.0e9
N_WARM = 20


def build_nc(D=1024, E=1024, SK=2048, QSH=1024, QB=512):
    import concourse.bass as bass
    import concourse.mybir as mybir
    import concourse.tile as tile
    from concourse import bacc

    f32 = mybir.dt.float32
    f32r = mybir.dt.float32r
    bf16 = mybir.dt.float16
    AF = mybir.ActivationFunctionType

    DT = D // P           # contraction tiles over model dim (8)
    ET = E // P           # enc tiles (8)
    KT = SK // P          # key tiles (16)
    NQB = QSH // QB       # q blocks (2)
    NKB = SK // QB        # key blocks of 512 (4)
    MQ = QB // P          # q subtiles per block (4)
    ND = D // QB          # out col blocks (2)
    HKT = KT // 2         # key tiles per half (8)
    ISCALE = 1.0 / float(np.sqrt(E))

    nc = bacc.Bacc(trn_type="TRN2")

    # ---- I/O ----
    qT = nc.dram_tensor("qT", [D, QSH], bf16, kind="ExternalInput")[:, :]
    kT = nc.dram_tensor("kT", [D, SK], bf16, kind="ExternalInput")[:, :]
    vT = nc.dram_tensor("vT", [D, SK], bf16, kind="ExternalInput")[:, :]
    wq = nc.dram_tensor("wq", [D, E], bf16, kind="ExternalInput")[:, :]
    wk = nc.dram_tensor("wk", [D, E], bf16, kind="ExternalInput")[:, :]
    wv = nc.dram_tensor("wv", [D, E], bf16, kind="ExternalInput")[:, :]
    ow = nc.dram_tensor("ow", [E, D], bf16, kind="ExternalInput")[:, :]
    mask_cols = nc.dram_tensor("mask_cols", [P, KT], f32, kind="ExternalInput")[:, :]
    ones_d = nc.dram_tensor("ones_d", [P, P], f32r, kind="ExternalInput")[:, :]
    bq_col = nc.dram_tensor("bq_col", [P, ET], f32, kind="ExternalInput")[:, :]
    bk_col = nc.dram_tensor("bk_col", [P, ET], f32, kind="ExternalInput")[:, :]
    bv_bc = nc.dram_tensor("bv_bc", [P, E], f32, kind="ExternalInput")[:, :]
    ob_bc = nc.dram_tensor("ob_bc", [P, D], f32, kind="ExternalInput")[:, :]
    out = nc.dram_tensor("out", [QSH, D], f32, kind="ExternalOutput")[:, :]

    qT_r = qT.rearrange("(t p) n -> p t n", p=P)   # [128, DT, QSH]
    kT_r = kT.rearrange("(t p) n -> p t n", p=P)
    vT_r = vT.rearrange("(t p) n -> p t n", p=P)
    wq_r = wq.rearrange("(t p) n -> p t n", p=P)   # [128, DT, E]
    wk_r = wk.rearrange("(t p) n -> p t n", p=P)
    wv_r = wv.rearrange("(t p) n -> p t n", p=P)
    ow_r = ow.rearrange("(t p) n -> p t n", p=P)   # [128, ET, D]

    def mm(ps, lhsT, rhs, start, stop):
        nc.tensor.matmul(ps, lhsT, rhs, start=start, stop=stop)

    with tile.TileContext(nc) as tc:
        # ---- persistent pools (entered in reverse order of death; LIFO) ----
        smalls_cm = tc.tile_pool(name="smalls", bufs=1)
        smalls = smalls_cm.__enter__()
        osb_cm = tc.tile_pool(name="osb", bufs=3)
        osb = osb_cm.__enter__()
        ctxp_cm = tc.tile_pool(name="ctxp", bufs=2)
        ctxp = ctxp_cm.__enter__()
        expp_cm = tc.tile_pool(name="expp", bufs=1)
        expp = expp_cm.__enter__()
        vpp_cm = tc.tile_pool(name="vpp", bufs=1)
        vpp = vpp_cm.__enter__()
        kpp_cm = tc.tile_pool(name="kpp", bufs=1)
        kpp = kpp_cm.__enter__()
        qpp_cm = tc.tile_pool(name="qpp", bufs=1)
        qpp = qpp_cm.__enter__()
        # alternating weight pools: wA holds wq then wv, wB holds wk then ow.
        wA_cm = tc.tile_pool(name="wA", bufs=1)
        wA = wA_cm.__enter__()
        wB_cm = tc.tile_pool(name="wB", bufs=1)
        wB = wB_cm.__enter__()
        # stream pools: S_A holds qT blocks then vT blocks, S_B holds kT.
        sA_cm = tc.tile_pool(name="sA", bufs=4)
        sA = sA_cm.__enter__()
        sB_cm = tc.tile_pool(name="sB", bufs=4)
        sB = sB_cm.__enter__()

        # one global PSUM pool: 8 bank tags, allocation order chosen so each
        # phase's first bank aliases the earliest-released predecessor.
        gps_cm = tc.tile_pool(name="gps", bufs=1, space="PSUM")
        gps = gps_cm.__enter__()

        # smalls on gpsimd SWDGE: keeps the HWDGE free for the wq stream.
        # Issued after the first qT blocks (Pool program order).
        qp = qpp.tile([P, ET, QSH], bf16, name="qp")
        kp = kpp.tile([P, ET, SK], bf16, name="kp")
        vp = vpp.tile([P, KT, E], bf16, name="vp")

        # wq per-t tiles (fine-grained: QP matmuls start on first arrival);
        # t0 lands as two separate half-tiles so the very first matmuls
        # only wait for a 128KB transfer
        wq_ts = []
        qt01 = sA.tile([P, 4, QB], bf16, tag="s4", name="qt01")
        wq0a = wA.tile([P, E // 2], bf16, tag="w0a", name="wq0a")
        nc.sync.dma_start(wq0a[:], wq_r[:, 0, :E // 2])
        wq0b = wA.tile([P, E // 2], bf16, tag="w0b", name="wq0b")
        nc.sync.dma_start(wq0b[:], wq_r[:, 0, E // 2:])
        wq_ts.append((wq0a, wq0b))
        for t in range(1, DT):
            w = wA.tile([P, E], bf16, tag=f"w{t}", name=f"wq{t}")
            nc.sync.dma_start(w[:], wq_r[:, t, :])
            wq_ts.append(w)
            if t == 3:
                nc.sync.dma_start(qt01[:], qT_r[:, 4:8, 0:QB])
        wk_ts = []
        for t in range(DT):
            w = wB.tile([P, E], bf16, tag=f"w{t}", name=f"wk{t}")
            nc.sync.dma_start(w[:], wk_r[:, t, :])
            wk_ts.append(w)
        # qh0 tb0 as per-t tiles (earliest possible first matmul), rest blocks
        qt0_ts = []
        for ti in range(4):
            qt = sA.tile([P, QB], bf16, tag="s", name=f"qt00_{ti}")
            nc.gpsimd.dma_start(qt[:], qT_r[:, ti, 0:QB])
            qt0_ts.append(qt)
        bq_t = smalls.tile([P, ET], f32, name="bqc")
        nc.gpsimd.dma_start(bq_t[:], bq_col)
        qt1_bs = []
        for tb in range(2):
            qb_t = sA.tile([P, 4, QB], bf16, tag="s4", name=f"qt1_{tb}")
            nc.gpsimd.dma_start(qb_t[:], qT_r[:, 4 * tb:4 * tb + 4, QB:2 * QB])
            qt1_bs.append(qb_t)
        ones_t = smalls.tile([P, P], f32r, name="ones")
        nc.gpsimd.dma_start(ones_t[:], ones_d)
        mask_t = smalls.tile([P, KT], f32, name="maskc")
        nc.gpsimd.dma_start(mask_t[:], mask_cols)
        bk_t = smalls.tile([P, ET], f32, name="bkc")
        nc.gpsimd.dma_start(bk_t[:], bk_col)
        bv_t = smalls.tile([P, E], f32, name="bvc")
        nc.gpsimd.dma_start(bv_t[:], bv_bc)
        ob_t = smalls.tile([P, D], f32, name="obc")
        nc.gpsimd.dma_start(ob_t[:], ob_bc)
        recip_ts = [smalls.tile([P, QB], f32, name=f"recip{i}") for i in range(NQB)]
        sacc_ts = [smalls.tile([P, QB], f32r, name=f"sacc{i}") for i in range(NQB)]

        # ---- warmup: keep the PE p-state ramp off the critical path ----
        with tc.tile_pool(name="warm", bufs=1) as wrm:
            dummy = wrm.tile([P, P], bf16, name="dummy")
            nc.vector.memset(dummy[:], 0.0)
            wps = gps.tile([P, P], f32, tag="b7", name="wps")
            for _ in range(N_WARM):
                mm(wps[:], dummy[:], dummy[:], True, True)

        # ---- QP: qp^T [E, QSH]; loop orders matched to DMA arrival ----
        # qh0 t0-3: t-outer (each arriving wq tile unlocks 8 matmuls);
        # qh0 t4-6: t-outer; t7: m-outer with stops so drains pipeline.
        def wq_sl(t, m):
            if t == 0:
                half = wq_ts[0][0] if m < 4 else wq_ts[0][1]
                return half[:, (m % 4) * P:(m % 4 + 1) * P]
            return wq_ts[t][:, m * P:(m + 1) * P]

        # four zero-cost data-dependent weight loads fill the PE wait queue
        # so no real matmul is dispatched (and mid-clock priced) inside the
        # p-state ramp window
        for _ in range(4):
            nc.tensor.ldweights(wq_ts[0][0][:, :P])
        pss0 = [gps.tile([P, QB], f32, tag=f"b{m}", name=f"qpps0_{m}")
                for m in range(ET)]
        for ti in range(4):
            for m in range(ET):
                mm(pss0[m][:], wq_sl(ti, m), qt0_ts[ti][:], ti == 0, False)
        for ti in range(3):
            t = 4 + ti
            for m in range(ET):
                mm(pss0[m][:], wq_sl(t, m), qt01[:, ti, :], False, False)
        for m in range(ET):
            mm(pss0[m][:], wq_sl(7, m), qt01[:, 3, :], False, True)
            nc.scalar.activation(qp[:, m, 0:QB], pss0[m][:], AF.Identity,
                                 bias=bq_t[:, m:m + 1], scale=ISCALE)
        # qh1: m-outer chains (all data resident), banks reused as drained
        for m in range(ET):
            ps = gps.tile([P, QB], f32, tag=f"b{m}", name=f"qpps1_{m}")
            for t in range(DT):
                mm(ps[:], wq_sl(t, m),
                   qt1_bs[t // 4][:, t % 4, :], t == 0, t == DT - 1)
            nc.scalar.activation(qp[:, m, QB:2 * QB], ps[:], AF.Identity,
                                 bias=bq_t[:, m:m + 1], scale=ISCALE)

        # ---- KP: kp^T [E, SK] (kT streamed per key block; m-outer chains) ----
        kcnt = 0
        for nb in range(NKB):
            kt_bs = []
            for tb in range(2):
                kt_b = sB.tile([P, 4, QB], bf16, tag="s", name=f"kt{nb}_{tb}")
                nc.gpsimd.dma_start(
                    kt_b[:], kT_r[:, 4 * tb:4 * tb + 4, nb * QB:(nb + 1) * QB])
                kt_bs.append(kt_b)
            for m in range(ET):
                ps = gps.tile([P, QB], f32, tag=f"b{kcnt % 3}",
                              name=f"kpps{nb}_{m}")
                kcnt += 1
                for t in range(DT):
                    mm(ps[:], wk_ts[t][:, m * P:(m + 1) * P],
                       kt_bs[t // 4][:, t % 4, :], t == 0, t == DT - 1)
                nc.scalar.activation(kp[:, m, nb * QB:(nb + 1) * QB],
                                     ps[:], AF.Identity,
                                     bias=bk_t[:, m:m + 1])

        # wv reuses wA slots (freed as QP's last reads complete); prefetches
        # during KP.  ow reuses wB slots; prefetches during VP.
        wv0a = wA.tile([P, E // 2], bf16, tag="w0a", name="wv0a")
        nc.sync.dma_start(wv0a[:], wv_r[:, 0, :E // 2])
        wv0b = wA.tile([P, E // 2], bf16, tag="w0b", name="wv0b")
        nc.sync.dma_start(wv0b[:], wv_r[:, 0, E // 2:])
        wv_ts = [(wv0a, wv0b)]
        for t in range(1, DT):
            w = wA.tile([P, E], bf16, tag=f"w{t}", name=f"wv{t}")
            nc.sync.dma_start(w[:], wv_r[:, t, :])
            wv_ts.append(w)

        def wv_sl(t, h):
            if t == 0:
                return wv_ts[0][h][:]
            return wv_ts[t][:, h * QB:(h + 1) * QB]

        # ---- VP: vp [SK, E] (vT streamed per key block; tile-outer chains) ----
        vcnt = 0
        for nb in range(NKB):
            vt_bs = []
            for tb in range(2):
                vt_b = sA.tile([P, 4, QB], bf16, tag="s4", name=f"vt{nb}_{tb}")
                nc.sync.dma_start(
                    vt_b[:], vT_r[:, 4 * tb:4 * tb + 4, nb * QB:(nb + 1) * QB])
                vt_bs.append(vt_b)
            for kbl in range(MQ):
                kb = nb * MQ + kbl
                for h in range(2):
                    ps = gps.tile([P, QB], f32, tag=f"b{3 + vcnt % 3}",
                                  name=f"vpps{kb}_{h}")
                    vcnt += 1
                    for t in range(DT):
                        mm(ps[:], vt_bs[t // 4][:, t % 4, kbl * P:(kbl + 1) * P],
                           wv_sl(t, h), t == 0, t == DT - 1)
                    nc.vector.tensor_add(vp[:, kb, h * QB:(h + 1) * QB],
                                         ps[:], bv_t[:, h * QB:(h + 1) * QB])

        ow_ts = []
        for e in range(ET):
            w = wB.tile([P, D], bf16, tag=f"w{e}", name=f"ow{e}")
            nc.sync.dma_start(w[:], ow_r[:, e, :])
            ow_ts.append(w)

        # ---- per q-block: logits/exp -> ctx -> out ----
        for qb in range(NQB):
            q0 = qb * QB
            expa = expp.tile([P, HKT, QB], bf16, tag="expa", name=f"expa{qb}")
            expb = expp.tile([P, HKT, QB], bf16, tag="expb", name=f"expb{qb}")
            exp_half = [expa, expb]
            # ctx in two halves so the out phase never waits for the last
            # normalize (whole-tile dependency granularity)
            ctxa = ctxp.tile([P, 4, QB], bf16, tag="ctxa", name=f"ctxa{qb}")
            ctxb = ctxp.tile([P, 4, QB], bf16, tag="ctxb", name=f"ctxb{qb}")
            sacc = sacc_ts[qb]

            # logits + exp; DVE accumulates softmax sums
            for kb in range(KT):
                ps = gps.tile([P, QB], f32, tag=f"b{kb % 3}", name=f"lg{qb}_{kb}")
                for e in range(ET):
                    mm(ps[:], kp[:, e, kb * P:(kb + 1) * P],
                       qp[:, e, q0:q0 + QB], e == 0, e == ET - 1)
                ex = exp_half[kb // HKT]
                nc.scalar.activation(ex[:, kb % HKT, :], ps[:], AF.Exp,
                                     bias=mask_t[:, kb:kb + 1])
                if kb == 0:
                    nc.vector.tensor_copy(sacc[:], ex[:, 0, :])
                else:
                    nc.vector.tensor_add(sacc[:], sacc[:],
                                         ex[:, kb % HKT, :])

            # ctx accumulation in half-key groups: the first group only
            # needs the first 8 exp tiles, so the PE never waits for the
            # last exp.  e-groups of 4 keep PSUM use at 4 banks (b3-b6).
            cps = {}
            for eg in range(2):
                es = range(4 * eg, 4 * eg + 4)
                for half in range(2):
                    ex = exp_half[half]
                    for e in es:
                        if half == 0:
                            cps[e] = gps.tile([P, QB], f32, tag=f"b{3 + e % 4}",
                                              name=f"c{qb}_{e}")
                        for ki in range(HKT):
                            kb = half * HKT + ki
                            mm(cps[e][:], vp[:, kb, e * P:(e + 1) * P],
                               ex[:, ki, :], kb == 0, kb == KT - 1)
                    if eg == 0 and half == 0:
                        # softmax denominator: ones-matmul + reciprocal
                        s_ps = gps.tile([P, QB], f32, tag="b7",
                                        name=f"sps{qb}")
                        mm(s_ps[:], ones_t[:], sacc[:], True, True)
                        nc.vector.reciprocal(recip_ts[qb][:], s_ps[:])
                    if half == 1:
                        for e in es:
                            ctx_half = ctxa if e < 4 else ctxb
                            nc.vector.tensor_mul(ctx_half[:, e % 4, :],
                                                 cps[e][:], recip_ts[qb][:])

            # out block: out[q0:q0+QB, :] = ctx^T.T @ ow + ob
            last_qb = qb == NQB - 1
            ocnt = 0
            for mq in range(MQ):
                for nd in range(ND):
                    rows = slice(q0 + mq * P, q0 + (mq + 1) * P)
                    last_tile = last_qb and mq == MQ - 1 and nd == ND - 1
                    if last_tile:
                        # final tile as two N=256 chains: the first half's
                        # store overlaps the second half's matmuls, so the
                        # kernel tail is one short store chain
                        for sub in range(2):
                            c0 = nd * QB + sub * (QB // 2)
                            ps = gps.tile([P, QB // 2], f32,
                                          tag=f"b{ocnt % 3}",
                                          name=f"o{qb}_{mq}_{nd}_{sub}")
                            ocnt += 1
                            for e in range(ET):
                                ctx_half = ctxa if e < 4 else ctxb
                                mm(ps[:],
                                   ctx_half[:, e % 4, mq * P:(mq + 1) * P],
                                   ow_ts[e][:, c0:c0 + QB // 2],
                                   e == 0, e == ET - 1)
                            ot = osb.tile([P, QB // 2], f32, tag="oth",
                                          name=f"ot{qb}_{mq}_{nd}_{sub}")
                            nc.vector.tensor_add(ot[:], ps[:],
                                                 ob_t[:, c0:c0 + QB // 2])
                            nc.sync.dma_start(out[rows, c0:c0 + QB // 2],
                                              ot[:])
                        continue
                    ps = gps.tile([P, QB], f32, tag=f"b{ocnt % 3}",
                                  name=f"o{qb}_{mq}_{nd}")
                    ocnt += 1
                    for e in range(ET):
                        ctx_half = ctxa if e < 4 else ctxb
                        mm(ps[:], ctx_half[:, e % 4, mq * P:(mq + 1) * P],
                           ow_ts[e][:, nd * QB:(nd + 1) * QB],
                           e == 0, e == ET - 1)
                    ot = osb.tile([P, QB], f32, tag="ot",
                                  name=f"ot{qb}_{mq}_{nd}")
                    nc.vector.tensor_add(ot[:], ps[:],
                                         ob_t[:, nd * QB:(nd + 1) * QB])
                    eng = (nc.sync if last_qb and mq == MQ - 1 else nc.gpsimd)
                    eng.dma_start(out[rows, nd * QB:(nd + 1) * QB], ot[:])

        gps_cm.__exit__(None, None, None)
        sB_cm.__exit__(None, None, None)
        sA_cm.__exit__(None, None, None)
        wB_cm.__exit__(None, None, None)
        wA_cm.__exit__(None, None, None)
        qpp_cm.__exit__(None, None, None)
        kpp_cm.__exit__(None, None, None)
        vpp_cm.__exit__(None, None, None)
        expp_cm.__exit__(None, None, None)
        ctxp_cm.__exit__(None, None, None)
        osb_cm.__exit__(None, None, None)
        smalls_cm.__exit__(None, None, None)

    nc.compile()
    return nc


def make_in_maps(v, k, q, mask, wq_w, wq_b, wk_w, wk_b, wv_w, wv_b, out_w, out_b,
                 n_cores=8, D=1024, E=1024, SK=2048, QSH=1024):
    """Host-side shard + layout prep (pure data movement + dtype cast)."""
    import ml_dtypes
    bf = np.float16
    ET = E // P
    KT = SK // P
    f = np.float32
    iscale = f(1.0 / np.sqrt(E))
    wq_bf = np.ascontiguousarray(np.asarray(wq_w, f).astype(bf))
    wk_bf = np.ascontiguousarray(np.asarray(wk_w, f).astype(bf))
    wv_bf = np.ascontiguousarray(np.asarray(wv_w, f).astype(bf))
    ow_bf = np.ascontiguousarray(np.asarray(out_w, f).astype(bf))
    bq_col = np.ascontiguousarray((np.asarray(wq_b, f) * iscale).reshape(ET, P).T)
    bk_col = np.ascontiguousarray(np.asarray(wk_b, f).reshape(ET, P).T)
    bv_bc = np.ascontiguousarray(np.broadcast_to(np.asarray(wv_b, f), (P, E)))
    ob_bc = np.ascontiguousarray(
        np.broadcast_to(np.asarray(out_b, f), (P, len(out_b))))
    ones_arr = np.ones((P, P), f)
    in_maps = []
    for c in range(n_cores):
        b, h = divmod(c, 2)
        qTc = np.ascontiguousarray(
            np.asarray(q[b, h * QSH:(h + 1) * QSH, :], f).T.astype(bf))
        kTc = np.ascontiguousarray(np.asarray(k[b], f).T.astype(bf))
        vTc = np.ascontiguousarray(np.asarray(v[b], f).T.astype(bf))
        mc = np.ascontiguousarray(
            (np.asarray(mask[b, 0], f) * np.float32(NEG)).reshape(KT, P).T)
        in_maps.append(dict(qT=qTc, kT=kTc, vT=vTc, mask_cols=mc,
                            ones_d=ones_arr,
                            wq=wq_bf, wk=wk_bf, wv=wv_bf, ow=ow_bf,
                            bq_col=bq_col, bk_col=bk_col,
                            bv_bc=bv_bc, ob_bc=ob_bc))
    return in_maps


_NC_CACHE = {}


def kernel(v, k, q, mask, wq_w, wq_b, wk_w, wk_b, wv_w, wv_b, out_w, out_b):
    from concourse.bass_utils import run_bass_kernel_spmd

    B, S, D = 4, 2048, 1024
    E, QSH = 1024, 1024
    if "nc" not in _NC_CACHE:
        _NC_CACHE["nc"] = build_nc(D=D, E=E, SK=S, QSH=QSH, QB=512)
    nc = _NC_CACHE["nc"]

    in_maps = make_in_maps(v, k, q, mask, wq_w, wq_b, wk_w, wk_b, wv_w, wv_b,
                           out_w, out_b, n_cores=8, D=D, E=E, SK=S, QSH=QSH)
    trace = bool(int(os.environ.get("BASS_KERNEL_TRACE", "0")))
    res = run_bass_kernel_spmd(nc, in_maps, core_ids=list(range(8)), trace=trace)
    if trace:
        print(f"HW exec time: {res.exec_time_ns} ns")
        _NC_CACHE["last_exec_time_ns"] = res.exec_time_ns
        _NC_CACHE["last_trace"] = res.instructions_and_trace

    outp = np.empty((B, S, D), np.float32)
    for c in range(8):
        b, h = divmod(c, 2)
        outp[b, h * QSH:(h + 1) * QSH, :] = res.results[c]["out"]
    return outp
